# revision 13
# baseline (speedup 1.0000x reference)
"""Distributed Trainium2 kernel for nn_CompareLoss (8 NeuronCores).

Math (validated against the reference):
  z = [strong; weak]  (2B x D), s = z / ||z||  (row-normalized)
  logits(i,j) = (s_i . s_j) / tau,  pos_i = logits(i, B+i) = logits(B+i, i)
  Every row r of the similarity matrix contributes  ln(S_r) - pos_r  where
    S_r = exp(pos_r) + sum_{j in C(r)} exp(logits(r, j))
  with column set C(r):
    - "positive" rows (strong_i / weak_{B+i}, i < P): C = all 2N negative rows
    - "negative" rows (i >= P):                       C = the P strong-positive rows
  loss = (sum over all 2B rows) / (2B).
  Logits are bounded by 1/tau, so no max-subtraction is needed in the LSE.

Sharding: data-parallel over the pair index i. Core c owns i in
[c*256,(c+1)*256) of the positives AND of the negatives -> 1024 rows/core,
perfectly balanced work. Each core receives the full column set
feature-major (z^T) with its own row blocks rotated to the front of each
region so one SPMD program (fixed offsets) serves all 8 cores. No
collectives: on this fabric an 8-rank collective has a ~7-20us floor,
far more than host-summing 8 scalar partials.

On-device pipeline (all aux matmuls in fp16 - fp32 matmuls run LOW_HIGH
double-pass on TRN2 and must be avoided):
  A) per 512-col chunk: square (gpsimd/DVE split) -> ones-matmul partition
     sum -> [1,w] psum; move to a compact [128,*] layout (ACT Ln-direct or
     DVE copy, then reshape DMA).
  B) one Exp(-0.5*ln(ssq) + 0.5*ln(1/tau)) -> fp16 recip-norms (rsqrt and
     the sqrt(1/tau) logit scale folded into one table-friendly pass;
     Ln/Exp phases are strictly separated so the ACT table loads 3x total).
  C) per chunk: PE ones-broadcast of the norms -> scale z^T -> fp16 ztn.
  Mains: 12 jobs of [128,2048]: 8 fp16 matmuls + one in-place Exp with
  fused row-sum (activation accum_out). ln(S)-pos reduced on-chip to one
  f32 partial per core; host adds 8 partials and divides by 2B.
"""

import numpy as np

B = 4096
D = 256
P = 2048
NCORES = 8
IC = P // NCORES          # 256 pair-indices per core (per pos/neg half)
NCOL = 3 * P + IC         # 6400 columns in zt: [sneg | wneg | spos | wp_c]

OFF_SNEG = 0
OFF_WNEG = P
OFF_SPOS = 2 * P
OFF_WP = 3 * P
# lhsT column offsets for the 8 m-tiles (128 rows each):
#   M1 (positive rows): sp0 sp1 wp0 wp1      M2 (negative rows): sn0 sn1 wn0 wn1
LHS_OFF = [OFF_SPOS, OFF_SPOS + 128, OFF_WP, OFF_WP + 128,
           OFF_SNEG, OFF_SNEG + 128, OFF_WNEG, OFF_WNEG + 128]
POS_COL = [0, 1, 0, 1, 2, 3, 2, 3]   # pos i-tile used by each m-tile

# normalization chunks: G1 = everything the M2 jobs + pos logits + all lhsT
# tiles need; G2 = the M1 rhs columns. Widths are multiples of 128.
G1_CHUNKS = [(4096, 512), (4608, 512), (5120, 512), (5632, 512), (6144, 256),
             (0, 256), (2048, 256)]
G2_CHUNKS = [(256, 512), (768, 512), (1280, 512), (1792, 256),
             (2304, 512), (2816, 512), (3328, 512), (3840, 256)]
CHUNKS = G1_CHUNKS + G2_CHUNKS
N_ACT_PATH = len(G1_CHUNKS)   # G1 chunks move psum->sbuf via fused ACT Ln,
                              # G2 chunks via DVE copy (ln'd later compactly)

_CACHE: dict = {}


def _build_nc():
    import concourse.bacc as bacc
    import concourse.tile as tile
    from concourse import mybir

    f32 = mybir.dt.float32
    f16 = mybir.dt.float16
    EXP = mybir.ActivationFunctionType.Exp
    LN = mybir.ActivationFunctionType.Ln
    AX = mybir.AxisListType.X
    ADD = mybir.AluOpType.add

    nc = bacc.Bacc("TRN2", target_bir_lowering=False, debug=False,
                   num_devices=NCORES)
    zt_d = nc.dram_tensor("zt", [D, NCOL], f32, kind="ExternalInput")
    tp_d = nc.dram_tensor("temp", [1, 1], f32, kind="ExternalInput")
    out_d = nc.dram_tensor("out", [1, 1], f32, kind="ExternalOutput")

    # column offset of each chunk inside the compact [128, 50] layout
    coffs, acc = [], 0
    for _, w in CHUNKS:
        coffs.append(acc)
        acc += w // 128
    NCC = acc  # 50

    with tile.TileContext(nc) as tc:
        with (
            tc.tile_pool(name="const", bufs=1) as constp,
            tc.tile_pool(name="big", bufs=1) as bigp,
            tc.tile_pool(name="work", bufs=3) as workp,
            tc.tile_pool(name="ps", bufs=2, space="PSUM") as psp,
        ):
            # ---------------- constants ----------------
            ones16_k = constp.tile([128, 1], f16)    # fp16 partition-sum
            nc.gpsimd.memset(ones16_k[:], 1.0)
            ones16_1 = constp.tile([1, 128], f16)    # fp16 partition-broadcast
            nc.gpsimd.memset(ones16_1[:], 1.0)
            ident16 = constp.tile([1, 1], f16)
            nc.gpsimd.memset(ident16[:], 1.0)
            ones_k = constp.tile([128, 1], f32)      # final f32 total-sum
            nc.gpsimd.memset(ones_k[:], 1.0)

            tsb = constp.tile([1, 1], f32)
            nc.sync.dma_start(tsb[:], tp_d[:])
            invt = constp.tile([1, 1], f32)
            nc.vector.reciprocal(invt[:], tsb[:])
            ln_invt = constp.tile([1, 1], f32)
            nc.scalar.activation(ln_invt[:], invt[:], LN)
            half_ln_invt = constp.tile([1, 1], f32)
            nc.scalar.mul(half_ln_invt[:], ln_invt[:], 0.5)
            # broadcast 0.5*ln(1/tau) to [128,1] via a K=1 matmul (NOT
            # gpsimd.partition_broadcast - its custom ucode forces a GPSIMD
            # library switch that stalls the engine for ~15us)
            hli16 = constp.tile([1, 1], f16)
            nc.vector.tensor_copy(hli16[:], half_ln_invt[:])
            bias_ps = psp.tile([128, 1], f32, tag="ps")
            nc.tensor.matmul(bias_ps[:], ones16_1[:], hli16[0:1, 0:1],
                             start=True, stop=True)
            bias_bc = constp.tile([128, 1], f32)     # 0.5*ln(1/tau) everywhere
            nc.vector.tensor_copy(bias_bc[:], bias_ps[:])

            # ---------------- load z^T (G1 columns first) ----------------
            # Groups are chained (group k waits on group k-1) so the DMA
            # engines complete them in chunk-processing order instead of
            # round-robining all ranges to a late joint finish.
            from concourse.tile_rust import add_dep_helper
            zt0 = bigp.tile([128, NCOL], f32)        # features 0:128
            zt1 = bigp.tile([128, NCOL], f32)        # features 128:256
            # group 0+1 (all of G1) run together; G2 groups chain behind
            dma_groups = [
                [(4096, 2304), (0, 256), (2048, 256)],
                [(256, 1792)],
                [(2304, 1792)],
            ]
            prev = []
            for grp in dma_groups:
                cur = []
                for s, w in grp:
                    cur.append(nc.sync.dma_start(zt0[:, s:s + w],
                                                 zt_d[0:128, s:s + w]))
                    cur.append(nc.sync.dma_start(zt1[:, s:s + w],
                                                 zt_d[128:D, s:s + w]))
                for a in cur:
                    for b in prev:
                        add_dep_helper(a.ins, b.ins, sync=True,
                                       reason="dma group ordering")
                prev = cur

            # ---------------- A/B/C: column normalization ----------------
            # Per chunk: squares -> ones-matmul partition sum -> [1,w] psum,
            # moved to a flat [1,*] staging buffer (G1 via fused ACT
            # Ln-from-psum, G2 via DVE copy). One reshape DMA per group
            # gives a compact [128,*] layout for the rsqrt, whose fp16
            # result is reshaped back and PE-broadcast per chunk for the
            # scale multiply. G1 completes first so the M2 similarity jobs
            # and pos logits can start while G2 is still streaming in.
            ztn0 = bigp.tile([128, NCOL], f16)
            ztn1 = bigp.tile([128, NCOL], f16)
            WA = sum(w for _, w in CHUNKS[:N_ACT_PATH])
            WB = sum(w for _, w in CHUNKS[N_ACT_PATH:])
            goffs, oa, ob = [], 0, 0
            for ci, (_, w) in enumerate(CHUNKS):
                if ci < N_ACT_PATH:
                    goffs.append(("A", oa)); oa += w
                else:
                    goffs.append(("B", ob)); ob += w
            flatA = constp.tile([1, WA], f32)
            flatB = constp.tile([1, WB], f32)

            def sumsq_chunk(ci):
                s, w = CHUNKS[ci]
                sq0 = workp.tile([128, 512], f16, tag="sq0", name=f"sq0_{ci}")
                sq1 = workp.tile([128, 512], f16, tag="sq1", name=f"sq1_{ci}")
                nc.gpsimd.tensor_mul(sq0[:, :w], zt0[:, s:s + w], zt0[:, s:s + w])
                nc.vector.tensor_mul(sq1[:, :w], zt1[:, s:s + w], zt1[:, s:s + w])
                ss_ps = psp.tile([1, 512], f32, tag="ps", name=f"ss{ci}")
                nc.tensor.matmul(ss_ps[0:1, :w], ones16_k[:], sq0[:, :w],
                                 start=True, stop=False)
                nc.tensor.matmul(ss_ps[0:1, :w], ones16_k[:], sq1[:, :w],
                                 start=False, stop=True)
                grp, go = goffs[ci]
                if grp == "A":
                    nc.scalar.activation(flatA[0:1, go:go + w],
                                         ss_ps[0:1, :w], LN)
                else:
                    nc.vector.tensor_copy(flatB[0:1, go:go + w],
                                          ss_ps[0:1, :w])

            def norm_chunk(ci, rnf):
                s, w = CHUNKS[ci]
                _, go = goffs[ci]
                bc_ps = psp.tile([128, 512], f32, tag="ps", name=f"bc{ci}")
                nc.tensor.matmul(bc_ps[:, :w], ones16_1[:],
                                 rnf[0:1, go:go + w], start=True, stop=True)
                nc.vector.tensor_mul(ztn0[:, s:s + w], zt0[:, s:s + w],
                                     bc_ps[:, :w])
                nc.vector.tensor_mul(ztn1[:, s:s + w], zt1[:, s:s + w],
                                     bc_ps[:, :w])

            # --- G1: sumsq -> rsqrt -> scale ---
            for ci in range(len(G1_CHUNKS)):
                sumsq_chunk(ci)
            # norm-reshape DMAs go on the Tensor sequencer's queue to avoid
            # head-of-line blocking behind the chained input DMAs on Sync
            LNa = constp.tile([128, WA // 128], f32)
            nc.scalar.dma_start(LNa[:], flatA[0:1, :])
            RNa = constp.tile([128, WA // 128], f16)
            nc.scalar.activation(RNa[:], LNa[:], EXP,
                                 scale=-0.5, bias=bias_bc[:, 0:1])
            rnfA = constp.tile([1, WA], f16)
            nc.scalar.dma_start(rnfA[0:1, :], RNa[:])
            for ci in range(len(G1_CHUNKS)):
                norm_chunk(ci, rnfA)

            # ---------------- pos logits (all columns in G1) ---------------
            pos_ps = psp.tile([1, 512], f32, tag="ps")
            for half, (ca, cb) in enumerate(
                    [(OFF_SPOS, OFF_WP), (OFF_SNEG, OFF_WNEG)]):
                pr0 = workp.tile([128, IC], f16, tag="pr0")
                pr1 = workp.tile([128, IC], f16, tag="pr1")
                nc.vector.tensor_mul(pr0[:], ztn0[:, ca:ca + IC],
                                     ztn0[:, cb:cb + IC])
                nc.vector.tensor_mul(pr1[:], ztn1[:, ca:ca + IC],
                                     ztn1[:, cb:cb + IC])
                o = half * 2 * 128
                nc.tensor.matmul(pos_ps[0:1, o:o + IC], ones16_k[:], pr0[:],
                                 start=True, stop=False)
                nc.tensor.matmul(pos_ps[0:1, o:o + IC], ones16_k[:], pr1[:],
                                 start=False, stop=True)
            pos_sb = constp.tile([1, 512], f32)
            nc.vector.tensor_copy(pos_sb[:], pos_ps[:])
            pos16 = constp.tile([1, 512], f16)
            nc.vector.tensor_copy(pos16[:], pos_sb[:])

            # transpose pos to per-partition layout via [1,128]x[1,1] matmuls
            P_mat = constp.tile([128, 8], f32)
            for t in range(4):
                pos_t = psp.tile([128, 1], f32, tag="ps", name=f"pt{t}")
                nc.tensor.matmul(pos_t[:], pos16[0:1, t * 128:(t + 1) * 128],
                                 ident16[0:1, 0:1], start=True, stop=True)
                for col in range(8):
                    if POS_COL[col] == t:
                        nc.vector.tensor_copy(P_mat[:, col:col + 1], pos_t[:])
            E_mat = constp.tile([128, 8], f32)
            nc.scalar.activation(E_mat[:], P_mat[:], EXP)

            # ---------------- main similarity jobs ----------------
            # 12 jobs of [128, 2048]: M1 m-tiles have 2 jobs (4096 cols),
            # M2 m-tiles have 1 (2048 cols). ACC col: M1 -> mt*2+j, M2 -> 8+mt.
            ACC = constp.tile([128, 12], f32)

            def main_job(mt, j, acccol):
                off = LHS_OFF[mt]
                js = (0 if mt < 4 else 2 * P) + j * 2048
                ps = psp.tile([128, 2048], f32, tag="ps", name=f"mm{acccol}")
                for h in range(4):
                    c0 = js + h * 512
                    nc.tensor.matmul(ps[:, h * 512:(h + 1) * 512],
                                     ztn0[:, off:off + 128],
                                     ztn0[:, c0:c0 + 512],
                                     start=True, stop=False)
                    nc.tensor.matmul(ps[:, h * 512:(h + 1) * 512],
                                     ztn1[:, off:off + 128],
                                     ztn1[:, c0:c0 + 512],
                                     start=False, stop=True)
                # in-place exp with fused row-sum
                nc.scalar.activation(ps[:], ps[:], EXP,
                                     accum_out=ACC[:, acccol:acccol + 1])

            # M2 jobs (need only G1) run while G2 streams in + normalizes
            for i in range(4):
                main_job(4 + i, 0, 8 + i)

            # --- G2: sumsq -> rsqrt -> scale ---
            for ci in range(len(G1_CHUNKS), len(CHUNKS)):
                sumsq_chunk(ci)
            SSb = constp.tile([128, WB // 128], f32)
            nc.scalar.dma_start(SSb[:], flatB[0:1, :])
            nc.scalar.activation(SSb[:], SSb[:], LN)      # in place
            RNb = constp.tile([128, WB // 128], f16)
            nc.scalar.activation(RNb[:], SSb[:], EXP,
                                 scale=-0.5, bias=bias_bc[:, 0:1])
            rnfB = constp.tile([1, WB], f16)
            nc.scalar.dma_start(rnfB[0:1, :], RNb[:])
            for ci in range(len(G1_CHUNKS), len(CHUNKS)):
                norm_chunk(ci, rnfB)

            for mt in range(4):
                main_job(mt, 0, mt * 2)
            for mt in range(4):
                main_job(mt, 1, mt * 2 + 1)

            # ---------------- reduce & finish ----------------
            RS = constp.tile([128, 8], f32)
            nc.vector.tensor_reduce(
                RS[:, 0:4], ACC[:, 0:8].rearrange("p (m j) -> p m j", j=2),
                axis=AX, op=ADD)
            nc.vector.tensor_copy(RS[:, 4:8], ACC[:, 8:12])
            S_mat = constp.tile([128, 8], f32)
            nc.vector.tensor_add(S_mat[:], RS[:], E_mat[:])
            LnS = constp.tile([128, 8], f32)
            nc.scalar.activation(LnS[:], S_mat[:], LN)
            Dif = constp.tile([128, 8], f32)
            nc.vector.tensor_sub(Dif[:], LnS[:], P_mat[:])
            part = constp.tile([128, 1], f32)
            nc.vector.tensor_reduce(part[:], Dif[:], axis=AX, op=ADD)
            tot_ps = psp.tile([1, 1], f32, tag="ps")
            nc.tensor.matmul(tot_ps[0:1, 0:1], ones_k[:], part[:],
                             start=True, stop=True)
            out_sb = constp.tile([1, 1], f32)
            nc.vector.tensor_copy(out_sb[:], tot_ps[:])
            nc.sync.dma_start(out_d[:], out_sb[:])

    nc.compile()
    return nc


def get_nc():
    if "nc" not in _CACHE:
        _CACHE["nc"] = _build_nc()
    return _CACHE["nc"]


def make_in_maps(strong: np.ndarray, weak: np.ndarray, temp: np.ndarray):
    """Host-side sharding: slice + rotate + transpose (pure data movement)."""
    in_maps = []
    for c in range(NCORES):
        r = c * IC
        sneg = np.roll(strong[P:B], -r, axis=0)   # own sn_c first
        wneg = np.roll(weak[P:B], -r, axis=0)     # own wn_c first
        spos = np.roll(strong[0:P], -r, axis=0)   # own sp_c first
        wp = weak[r:r + IC]
        zt = np.ascontiguousarray(
            np.concatenate([sneg, wneg, spos, wp], axis=0).T)
        in_maps.append({"zt": zt, "temp": temp})
    return in_maps


def kernel(inputs, strong_inputs, targets, num_pos, temperature):
    assert int(num_pos) == P
    strong = np.ascontiguousarray(np.asarray(strong_inputs, dtype=np.float32))
    weak = np.ascontiguousarray(np.asarray(inputs, dtype=np.float32))
    temp = np.asarray(temperature, dtype=np.float32).reshape(1, 1)

    from concourse.bass_utils import run_bass_kernel_spmd

    nc = get_nc()
    in_maps = make_in_maps(strong, weak, temp)
    res = run_bass_kernel_spmd(nc, in_maps, core_ids=list(range(NCORES)))
    total = sum(float(np.asarray(r["out"]).reshape(-1)[0])
                for r in res.results)
    return np.float32(total / (2 * B))


# revision 45
# speedup vs baseline: 1.2746x; 1.2746x over previous
"""Distributed Trainium2 kernel for nn_CompareLoss (8 NeuronCores).

Math (validated against the reference):
  z = [strong; weak]  (2B x D), s = z / ||z||  (row-normalized)
  logits(i,j) = (s_i . s_j) / tau,  pos_i = logits(i, B+i) = logits(B+i, i)
  Every row r of the similarity matrix contributes  ln(S_r) - pos_r  where
    S_r = exp(pos_r) + sum_{j in C(r)} exp(logits(r, j))
  with column set C(r):
    - "positive" rows (strong_i / weak_{B+i}, i < P): C = all 2N negative rows
    - "negative" rows (i >= P):                       C = the P strong-positive rows
  loss = (sum over all 2B rows) / (2B).
  Logits are bounded by 1/tau, so no max-subtraction is needed in the LSE.

Sharding: data-parallel over the pair index i. Core c owns i in
[c*256,(c+1)*256) of the positives AND of the negatives -> 1024 rows/core,
perfectly balanced work. Each core receives the full column set
feature-major (z^T) with its own row blocks rotated to the front of each
region so one SPMD program (fixed offsets) serves all 8 cores. No
collectives: on this fabric an 8-rank collective has a ~7-20us floor,
far more than host-summing 8 scalar partials.

On-device pipeline (all aux matmuls in fp16 - fp32 matmuls run LOW_HIGH
double-pass on TRN2 and must be avoided):
  A) column sum-of-squares: per 512-col chunk square z^T (G1 chunks on the
     vector engine, G2 on gpsimd) and partition-sum with a fp16 ones-matmul
     into batched [1,2048] psum tiles; one fused ACT Ln moves each batch
     psum->sbuf into a flat [1,W] staging buffer.
  B) rn = exp(-0.5*ln(ssq) + 0.5*ln(1/tau)) evaluated directly on the flat
     buffer (a ~3us single-partition ACT pass beats any reshape-DMA round
     trip); the rsqrt and the sqrt(1/tau) logit scale fold into one pass,
     and Ln/Exp phases are separated so the ACT table loads stay rare.
  C) G1 columns: per-chunk PE ones-broadcast of rn -> psum -> vector-engine
     scale multiply -> fp16 ztn (lowest latency, unblocks the M2 jobs).
     G2 columns: rn broadcast via a stride-0 DRAM-read DMA into SBUF, then
     two large multiplies per k-half split across vector/gpsimd (PSUM-free,
     so it cannot contend with the main jobs' psum slots).
  Mains: 12 jobs of [128,2048]: 8 fp16 matmuls + one Exp with fused
  row-sum (activation accum_out). ln(S)-pos is reduced on-chip to a single
  f32 partial per core; the host adds 8 partials and divides by 2B.
"""

import numpy as np

B = 4096
D = 256
P = 2048
NCORES = 8
IC = P // NCORES          # 256 pair-indices per core (per pos/neg half)
NCOL = 3 * P + IC         # 6400 columns in zt: [sneg | wneg | spos | wp_c]

OFF_SNEG = 0
OFF_WNEG = P
OFF_SPOS = 2 * P
OFF_WP = 3 * P
# lhsT column offsets for the 8 m-tiles (128 rows each):
#   M1 (positive rows): sp0 sp1 wp0 wp1      M2 (negative rows): sn0 sn1 wn0 wn1
LHS_OFF = [OFF_SPOS, OFF_SPOS + 128, OFF_WP, OFF_WP + 128,
           OFF_SNEG, OFF_SNEG + 128, OFF_WNEG, OFF_WNEG + 128]
POS_COL = [0, 1, 0, 1, 2, 3, 2, 3]   # pos i-tile used by each m-tile

# normalization chunks: G1 = everything the M2 jobs + pos logits + all lhsT
# tiles need; G2 = the M1 rhs columns. Widths are multiples of 128.
G1_CHUNKS = [(4096, 512), (4608, 512), (5120, 512), (5632, 512), (6144, 256),
             (0, 256), (2048, 256)]
G2_CHUNKS = [(256, 512), (768, 512), (1280, 512), (1792, 256),
             (2304, 512), (2816, 512), (3328, 512), (3840, 256)]
CHUNKS = G1_CHUNKS + G2_CHUNKS
N_ACT_PATH = len(G1_CHUNKS)   # G1 chunks move psum->sbuf via fused ACT Ln,
                              # G2 chunks via DVE copy (ln'd later compactly)

_CACHE: dict = {}


def _build_nc():
    import concourse.bacc as bacc
    import concourse.tile as tile
    from concourse import mybir

    f32 = mybir.dt.float32
    f16 = mybir.dt.float16
    EXP = mybir.ActivationFunctionType.Exp
    LN = mybir.ActivationFunctionType.Ln
    AX = mybir.AxisListType.X
    ADD = mybir.AluOpType.add

    nc = bacc.Bacc("TRN2", target_bir_lowering=False, debug=False,
                   num_devices=NCORES)
    zt_d = nc.dram_tensor("zt", [D, NCOL], f32, kind="ExternalInput")
    tp_d = nc.dram_tensor("temp", [1, 1], f32, kind="ExternalInput")
    out_d = nc.dram_tensor("out", [1, 1], f32, kind="ExternalOutput")

    # column offset of each chunk inside the compact [128, 50] layout
    coffs, acc = [], 0
    for _, w in CHUNKS:
        coffs.append(acc)
        acc += w // 128
    NCC = acc  # 50

    with tile.TileContext(nc) as tc:
        with (
            tc.tile_pool(name="const", bufs=1) as constp,
            tc.tile_pool(name="big", bufs=1) as bigp,
            tc.tile_pool(name="work", bufs=3) as workp,
            tc.tile_pool(name="esc", bufs=2) as escp,
            tc.tile_pool(name="dram", bufs=1, space="DRAM") as dramp,
            tc.tile_pool(name="ps", bufs=2, space="PSUM") as psp,
        ):
            # ---------------- constants ----------------
            ones16_k = constp.tile([128, 1], f16)    # fp16 partition-sum
            nc.gpsimd.memset(ones16_k[:], 1.0)
            ones16_1 = constp.tile([1, 128], f16)    # fp16 partition-broadcast
            nc.gpsimd.memset(ones16_1[:], 1.0)
            ident16 = constp.tile([1, 1], f16)
            nc.gpsimd.memset(ident16[:], 1.0)
            ones_k = constp.tile([128, 1], f32)      # final f32 total-sum
            nc.gpsimd.memset(ones_k[:], 1.0)

            tsb = constp.tile([1, 1], f32)
            nc.sync.dma_start(tsb[:], tp_d[:])
            invt = constp.tile([1, 1], f32)
            nc.vector.reciprocal(invt[:], tsb[:])
            ln_invt = constp.tile([1, 1], f32)
            nc.scalar.activation(ln_invt[:], invt[:], LN)
            half_ln_invt = constp.tile([1, 1], f32)
            nc.scalar.mul(half_ln_invt[:], ln_invt[:], 0.5)
            # broadcast 0.5*ln(1/tau) to [128,1] via a K=1 matmul (NOT
            # gpsimd.partition_broadcast - its custom ucode forces a GPSIMD
            # library switch that stalls the engine for ~15us)
            hli16 = constp.tile([1, 1], f16)
            nc.vector.tensor_copy(hli16[:], half_ln_invt[:])
            bias_ps = psp.tile([128, 1], f32, tag="ps")
            nc.tensor.matmul(bias_ps[:], ones16_1[:], hli16[0:1, 0:1],
                             start=True, stop=True)
            bias_bc = constp.tile([128, 1], f32)     # 0.5*ln(1/tau) everywhere
            nc.vector.tensor_copy(bias_bc[:], bias_ps[:])

            # ---------------- load z^T (G1 columns first) ----------------
            # Groups are chained (group k waits on group k-1) so the DMA
            # engines complete them in chunk-processing order instead of
            # round-robining all ranges to a late joint finish.
            from concourse.tile_rust import add_dep_helper
            zt0 = bigp.tile([128, NCOL], f32)        # features 0:128
            zt1 = bigp.tile([128, NCOL], f32)        # features 128:256
            # group 0 (all of G1) runs at full bandwidth; G2 chains behind
            dma_groups = [
                [(4096, 2304), (0, 256), (2048, 256)],
                [(256, 1792), (2304, 1792)],
            ]
            prev = []
            for grp in dma_groups:
                cur = []
                for s, w in grp:
                    cur.append(nc.sync.dma_start(zt0[:, s:s + w],
                                                 zt_d[0:128, s:s + w]))
                    cur.append(nc.sync.dma_start(zt1[:, s:s + w],
                                                 zt_d[128:D, s:s + w]))
                for a in cur:
                    for b in prev:
                        add_dep_helper(a.ins, b.ins, sync=True,
                                       reason="dma group ordering")
                prev = cur

            # ---------------- A/B/C: column normalization ----------------
            # Per chunk: squares -> ones-matmul partition sum -> [1,w] psum,
            # moved to a flat [1,*] staging buffer (G1 via fused ACT
            # Ln-from-psum, G2 via DVE copy). One reshape DMA per group
            # gives a compact [128,*] layout for the rsqrt, whose fp16
            # result is reshaped back and PE-broadcast per chunk for the
            # scale multiply. G1 completes first so the M2 similarity jobs
            # and pos logits can start while G2 is still streaming in.
            ztn0 = bigp.tile([128, NCOL], f16)
            ztn1 = bigp.tile([128, NCOL], f16)
            WA = sum(w for _, w in CHUNKS[:N_ACT_PATH])
            WB = sum(w for _, w in CHUNKS[N_ACT_PATH:])
            goffs, oa, ob = [], 0, 0
            for ci, (_, w) in enumerate(CHUNKS):
                if ci < N_ACT_PATH:
                    goffs.append(("A", oa)); oa += w
                else:
                    goffs.append(("B", ob)); ob += w
            flatA = constp.tile([1, WA], f32)
            flatB = constp.tile([1, WB], f32)

            def sumsq_batch(cis):
                """sumsq for a run of chunks into ONE [1, <=2048] psum tile
                (one pool-slot allocation + one big Ln instead of per-chunk
                ones - the 2-slot psum rotation is a serializer otherwise)."""
                grp, go0 = goffs[cis[0]]
                flat = flatA if grp == "A" else flatB
                wtot = sum(CHUNKS[ci][1] for ci in cis)
                assert wtot <= 2048
                ss_ps = psp.tile([1, 2048], f32, tag="ps",
                                 name=f"ssb{cis[0]}")
                o = 0
                for ci in cis:
                    s, w = CHUNKS[ci]
                    # G1 squares both on DVE (earliest data, keeps its FIFO
                    # short ahead of the C(G1) multiplies); G2 both on GPSIMD
                    eng = nc.vector if grp == "A" else nc.gpsimd
                    sq0 = workp.tile([128, 512], f16, tag="sq0",
                                     name=f"sq0_{ci}")
                    sq1 = workp.tile([128, 512], f16, tag="sq1",
                                     name=f"sq1_{ci}")
                    eng.tensor_mul(sq0[:, :w], zt0[:, s:s + w],
                                   zt0[:, s:s + w])
                    eng.tensor_mul(sq1[:, :w], zt1[:, s:s + w],
                                   zt1[:, s:s + w])
                    nc.tensor.matmul(ss_ps[0:1, o:o + w], ones16_k[:],
                                     sq0[:, :w], start=True, stop=False)
                    nc.tensor.matmul(ss_ps[0:1, o:o + w], ones16_k[:],
                                     sq1[:, :w], start=False, stop=True)
                    o += w
                # fused psum->sbuf move + ln on the scalar engine
                nc.scalar.activation(flat[0:1, go0:go0 + wtot],
                                     ss_ps[0:1, :wtot], LN)

            def norm_chunk(ci, rnflat):
                s, w = CHUNKS[ci]
                _, go = goffs[ci]
                bc_ps = psp.tile([128, 512], f32, tag="ps", name=f"bc{ci}")
                nc.tensor.matmul(bc_ps[:, :w], ones16_1[:],
                                 rnflat[0:1, go:go + w], start=True, stop=True)
                nc.vector.tensor_mul(ztn0[:, s:s + w], zt0[:, s:s + w],
                                     bc_ps[:, :w])
                nc.vector.tensor_mul(ztn1[:, s:s + w], zt1[:, s:s + w],
                                     bc_ps[:, :w])

            # --- G1: sumsq -> rsqrt-on-flat -> scale; unblocks M2 early ---
            sumsq_batch([0, 1, 2, 3])
            sumsq_batch([4, 5, 6])
            # --- G2 sumsq follows immediately ---
            sumsq_batch([7, 8, 9, 10])
            sumsq_batch([11, 12, 13, 14])
            # rn = exp(-0.5*ln(ssq)+0.5*ln(1/tau)) computed directly on the
            # flat [1,W] buffer: one single-partition ACT pass costs ~2.5us,
            # far less than the reshape-DMA round trip it replaces
            rnflatA = constp.tile([1, WA], f16)
            nc.scalar.activation(rnflatA[0:1, :], flatA[0:1, :], EXP,
                                 scale=-0.5, bias=bias_bc[0:1, 0:1])
            # M2-lhsT chunks (sn, wn) then its rhs; c4 (wp, only needed by
            # M1/pos) last so it absorbs the late G2-ssb slot release
            for ci in [5, 6, 0, 4, 1, 2, 3]:
                norm_chunk(ci, rnflatA)

            # ---------------- pos logits (all columns in G1) ---------------
            pos_ps = psp.tile([1, 512], f32, tag="ps")
            for half, (ca, cb) in enumerate(
                    [(OFF_SPOS, OFF_WP), (OFF_SNEG, OFF_WNEG)]):
                pr0 = workp.tile([128, IC], f16, tag="pr0")
                pr1 = workp.tile([128, IC], f16, tag="pr1")
                nc.vector.tensor_mul(pr0[:], ztn0[:, ca:ca + IC],
                                     ztn0[:, cb:cb + IC])
                nc.vector.tensor_mul(pr1[:], ztn1[:, ca:ca + IC],
                                     ztn1[:, cb:cb + IC])
                o = half * 2 * 128
                nc.tensor.matmul(pos_ps[0:1, o:o + IC], ones16_k[:], pr0[:],
                                 start=True, stop=False)
                nc.tensor.matmul(pos_ps[0:1, o:o + IC], ones16_k[:], pr1[:],
                                 start=False, stop=True)
            pos_sb = constp.tile([1, 512], f32)
            nc.vector.tensor_copy(pos_sb[:], pos_ps[:])
            pos16 = constp.tile([1, 512], f16)
            nc.vector.tensor_copy(pos16[:], pos_sb[:])

            # transpose pos to per-partition layout via [1,128]x[1,1] matmuls
            P_mat = constp.tile([128, 8], f32)
            for t in range(4):
                pos_t = psp.tile([128, 1], f32, tag="ps", name=f"pt{t}")
                nc.tensor.matmul(pos_t[:], pos16[0:1, t * 128:(t + 1) * 128],
                                 ident16[0:1, 0:1], start=True, stop=True)
                for col in range(8):
                    if POS_COL[col] == t:
                        nc.vector.tensor_copy(P_mat[:, col:col + 1], pos_t[:])

            # ---------------- main similarity jobs ----------------
            # 12 jobs of [128, 2048]: M1 m-tiles have 2 jobs (4096 cols),
            # M2 m-tiles have 1 (2048 cols). ACC col: M1 -> mt*2+j, M2 -> 8+mt.
            ACC = constp.tile([128, 12], f32)

            def main_job(mt, j, acccol):
                off = LHS_OFF[mt]
                js = (0 if mt < 4 else 2 * P) + j * 2048
                ps = psp.tile([128, 2048], f32, tag="ps", name=f"mm{acccol}")
                for h in range(4):
                    c0 = js + h * 512
                    nc.tensor.matmul(ps[:, h * 512:(h + 1) * 512],
                                     ztn0[:, off:off + 128],
                                     ztn0[:, c0:c0 + 512],
                                     start=True, stop=False)
                    nc.tensor.matmul(ps[:, h * 512:(h + 1) * 512],
                                     ztn1[:, off:off + 128],
                                     ztn1[:, c0:c0 + 512],
                                     start=False, stop=True)
                # exp with fused row-sum (SBUF dst: in-place psum writes
                # contend with the psum read port)
                esc = escp.tile([128, 2048], f16, tag="esc",
                                name=f"esc{acccol}")
                nc.scalar.activation(esc[:], ps[:], EXP,
                                     accum_out=ACC[:, acccol:acccol + 1])

            # --- G2 rsqrt + broadcast via DRAM (PSUM-free so it cannot
            # contend with the main jobs' psum slots; enables gpsimd mults)
            rnflatB = constp.tile([1, WB], f16)
            nc.scalar.activation(rnflatB[0:1, :], flatB[0:1, :], EXP,
                                 scale=-0.5, bias=bias_bc[0:1, 0:1])
            rnfB = dramp.tile([1, WB], f16)
            nc.scalar.dma_start(rnfB[0:1, :], rnflatB[0:1, :])
            rnb = bigp.tile([128, WB], f16)   # cols 256:2048 | 2304:4096
            for fo, w in [(0, 1792), (1792, 1792)]:
                nc.sync.dma_start(rnb[:, fo:fo + w],
                                  rnfB[0:1, fo:fo + w].to_broadcast((128, w)))

            # M2 jobs (need only G1 columns) overlap C(G2)
            for i in range(4):
                main_job(4 + i, 0, 8 + i)
            # C(G2): two large scale-multiplies per k-half, split DVE/gpsimd,
            # interleaved with the M1 jobs that consume them
            nc.vector.tensor_mul(ztn0[:, 256:2048], zt0[:, 256:2048],
                                 rnb[:, 0:1792])
            nc.gpsimd.tensor_mul(ztn1[:, 256:2048], zt1[:, 256:2048],
                                 rnb[:, 0:1792])
            for mt in range(4):
                main_job(mt, 0, mt * 2)
            nc.vector.tensor_mul(ztn0[:, 2304:4096], zt0[:, 2304:4096],
                                 rnb[:, 1792:3584])
            nc.vector.tensor_mul(ztn1[:, 2304:4096], zt1[:, 2304:4096],
                                 rnb[:, 1792:3584])
            for mt in range(4):
                main_job(mt, 1, mt * 2 + 1)

            # ---------------- reduce & finish ----------------
            # E_mat emitted here so its ACT slot doesn't head-of-line-block
            # the G2 Ln ops behind the pos-logit dependency
            E_mat = constp.tile([128, 8], f32)
            nc.scalar.activation(E_mat[:], P_mat[:], EXP)
            RS = constp.tile([128, 8], f32)
            nc.vector.tensor_reduce(
                RS[:, 0:4], ACC[:, 0:8].rearrange("p (m j) -> p m j", j=2),
                axis=AX, op=ADD)
            nc.vector.tensor_copy(RS[:, 4:8], ACC[:, 8:12])
            S_mat = constp.tile([128, 8], f32)
            nc.vector.tensor_add(S_mat[:], RS[:], E_mat[:])
            LnS = constp.tile([128, 8], f32)
            nc.scalar.activation(LnS[:], S_mat[:], LN)
            Dif = constp.tile([128, 8], f32)
            nc.vector.tensor_sub(Dif[:], LnS[:], P_mat[:])
            part = constp.tile([128, 1], f32)
            nc.vector.tensor_reduce(part[:], Dif[:], axis=AX, op=ADD)
            tot_ps = psp.tile([1, 1], f32, tag="ps")
            nc.tensor.matmul(tot_ps[0:1, 0:1], ones_k[:], part[:],
                             start=True, stop=True)
            out_sb = constp.tile([1, 1], f32)
            nc.vector.tensor_copy(out_sb[:], tot_ps[:])
            nc.sync.dma_start(out_d[:], out_sb[:])

    nc.compile()
    return nc


def get_nc():
    if "nc" not in _CACHE:
        _CACHE["nc"] = _build_nc()
    return _CACHE["nc"]


def make_in_maps(strong: np.ndarray, weak: np.ndarray, temp: np.ndarray):
    """Host-side sharding: slice + rotate + transpose (pure data movement)."""
    in_maps = []
    for c in range(NCORES):
        r = c * IC
        sneg = np.roll(strong[P:B], -r, axis=0)   # own sn_c first
        wneg = np.roll(weak[P:B], -r, axis=0)     # own wn_c first
        spos = np.roll(strong[0:P], -r, axis=0)   # own sp_c first
        wp = weak[r:r + IC]
        zt = np.ascontiguousarray(
            np.concatenate([sneg, wneg, spos, wp], axis=0).T)
        in_maps.append({"zt": zt, "temp": temp})
    return in_maps


def kernel(inputs, strong_inputs, targets, num_pos, temperature):
    assert int(num_pos) == P
    strong = np.ascontiguousarray(np.asarray(strong_inputs, dtype=np.float32))
    weak = np.ascontiguousarray(np.asarray(inputs, dtype=np.float32))
    temp = np.asarray(temperature, dtype=np.float32).reshape(1, 1)

    from concourse.bass_utils import run_bass_kernel_spmd

    nc = get_nc()
    in_maps = make_in_maps(strong, weak, temp)
    res = run_bass_kernel_spmd(nc, in_maps, core_ids=list(range(NCORES)))
    total = sum(float(np.asarray(r["out"]).reshape(-1)[0])
                for r in res.results)
    return np.float32(total / (2 * B))
